# revision 50
# baseline (speedup 1.0000x reference)
"""MultiHeadAttention Trainium2 kernel (8 NeuronCores).

Sharding: core c -> (batch b = c//2, head-group g = c%2) of the 12 heads.
Each core computes attention for its 6 heads of one batch element and a
partial projection; the host sums the two head-group partials per batch
element and adds the effective proj bias (proj_b + bv @ proj_w; the v bias
is additive after softmax because attention rows sum to 1).

Per-core dataflow (bf16 datapath, fp8 DoubleRow scores):
  x bf16 [S,768] --PE-transpose--> xT bf16 [128,6,S]
  q/k psum f32 = wqk_bf16^T @ xT; DVE adds bias and converts to fp8 in a
    zero-padded DoubleRow layout q8/k8 [64|64 part, pair, 2, S]
  v bf16 seq-major vsl [sk, skpair, j, head, 65] (+ ones col for denom)
  scores[sk,sq] = DoubleRow fp8 matmul (contraction 64 + 64 zeros)
  pt = exp(scores/8): split ACT (exact) / Pool / DVE (Schraudolph bits)
  av[sq,2,65] += pt-chunk^T @ v    (bf16, psum accum; col 64 = denom)
  ao_n = av / denom (broadcast divide, DVE), PE-transpose to aoT [384,S]
  yT[768,S] = wp_bf16^T @ aoT      (partial projection, host sums pairs)
"""
import sys

sys.path.insert(0, "/opt/trn_rl_repo")

import numpy as np

import concourse.bass as bass
import concourse.mybir as mybir
import concourse.tile as tile
from concourse import bacc
from concourse.bass_utils import run_bass_kernel_spmd
from concourse.masks import make_identity

F32 = mybir.dt.float32
BF16 = mybir.dt.bfloat16
U16 = mybir.dt.uint16
FP8 = mybir.dt.float8e4
EXP = mybir.ActivationFunctionType.Exp
COPY_FN = mybir.ActivationFunctionType.Identity
ADD = mybir.AluOpType.add
MULT = mybir.AluOpType.mult
DIV = mybir.AluOpType.divide
DR = mybir.MatmulPerfMode.DoubleRow

HID = 768
D = 64  # head dim
LHEADS = 6  # heads per core
PAIRS = 3

LOG2E = 1.4426950408889634
# Schraudolph-in-bf16-bits: n = x*0.125*128*log2e + (16256 + c); floor via
# the executor's f32->u16 cast. c=-6.85 calibrated for min rms vs exp().
SCHR_MUL = 0.125 * 128.0 * LOG2E
SCHR_ADD = 16256.0 - 6.85

# exp engine split: per block of 16 exp units (1024 rows each).
# GPSIMD (Pool) cannot access PSUM on TRN2, so only ACT and DVE apply.
EXP_PATTERN = (
    "act", "dve", "act", "act", "dve", "act", "dve", "act",
    "act", "dve", "act", "dve", "act", "act", "dve", "act",
)


def build_nc(S: int, taps: bool = False):
    nc = bacc.Bacc("TRN2", target_bir_lowering=False, debug=False)
    NSEQ = S // 128  # seq chunks of 128
    NBLK = S // 512  # seq blocks of 512
    NPAIR = NSEQ // 2  # sk chunk pairs
    XG = 4  # x DMA chunk groups
    NXG = NSEQ // XG

    x = nc.dram_tensor("x", [S, HID], BF16, kind="ExternalInput")
    wqk = nc.dram_tensor("wqk", [HID, 768], BF16, kind="ExternalInput")
    wv = nc.dram_tensor("wv", [HID, 384], BF16, kind="ExternalInput")
    bqk = nc.dram_tensor("bqk", [768], F32, kind="ExternalInput")
    wp = nc.dram_tensor("wp", [384, HID], BF16, kind="ExternalInput")
    z8 = nc.dram_tensor("z8", [128, 2048], FP8, kind="ExternalInput")
    yT = nc.dram_tensor("yT", [HID, S], F32, kind="ExternalOutput")
    if taps:
        t_xT = nc.dram_tensor("t_xT", [128, 6 * S], BF16, kind="ExternalOutput")
        t_q8 = nc.dram_tensor("t_q8", [128, 2 * S], FP8, kind="ExternalOutput")
        t_k8 = nc.dram_tensor("t_k8", [128, 2 * S], FP8, kind="ExternalOutput")
        t_v = nc.dram_tensor("t_v", [128, NSEQ // 2 * 2 * 6 * (D + 1)], BF16, kind="ExternalOutput")
        t_pt = nc.dram_tensor("t_pt", [128, 16 * 2 * 512], BF16, kind="ExternalOutput")
        t_ao = nc.dram_tensor("t_ao", [128, PAIRS * S], BF16, kind="ExternalOutput")
        t_aon = nc.dram_tensor("t_aon", [128, 2 * D], BF16, kind="ExternalOutput")
        t_qp = nc.dram_tensor("t_qp", [128, 2, 512], F32, kind="ExternalOutput")

    with tile.TileContext(nc) as tc:
        with (
            tc.tile_pool(name="const", bufs=1) as cp,
            tc.tile_pool(name="wts", bufs=1) as wpool,
            tc.tile_pool(name="qk8", bufs=1) as qk8p,
            tc.tile_pool(name="ao", bufs=1) as aop,
            tc.tile_pool(name="ps", bufs=1, space="PSUM") as ps,
        ):
            identf = cp.tile([128, 128], F32, tag="identf")
            make_identity(nc, identf[:])
            ident = cp.tile([128, 128], BF16, tag="ident")
            nc.vector.tensor_copy(ident[:], identf[:])
            # q/k bias, feature-major [128, 6] (chunk c: c<3 q, c>=3 k)
            bqk_sb = cp.tile([128, 6], F32, tag="bqk")
            nc.sync.dma_start(bqk_sb[:], bqk[:].rearrange("(c p) -> p c", p=128))
            # load the exp ACT table off the critical path
            warm = cp.tile([1, 16], F32, tag="warm")
            nc.vector.memset(warm[:], 1.0)
            nc.scalar.activation(warm[:], warm[:], EXP, bias=0.0, scale=0.0)

            # fp8 q/k, zero-padded DoubleRow layout, one tile per pair:
            # [128, 2, S]; partition half = head-of-pair, dim1 = j (j=1
            # stays zero) so [64*hi:64*hi+64, :, a:b] is a DR operand.
            q8_tiles = {}
            k8_tiles = {}

            def get_qk8(p):
                if p not in q8_tiles:
                    q8_p = qk8p.tile([128, 2, S], FP8, tag="q8", bufs=3,
                                     name=f"q8_{p}")
                    k8_p = qk8p.tile([128, 2, S], FP8, tag="k8", bufs=3,
                                     name=f"k8_{p}")
                    nc.sync.dma_start(q8_p[:, 1, :], z8[:])
                    nc.sync.dma_start(k8_p[:, 1, :], z8[:])
                    q8_tiles[p] = q8_p
                    k8_tiles[p] = k8_p
                return q8_tiles[p], k8_tiles[p]

            aoT = aop.tile([128, PAIRS, S], BF16, tag="aoT")
            yT_ap = yT[:].rearrange("(c p) s -> p c s", p=128)

            with tc.tile_pool(name="xT", bufs=1) as xtp, \
                 tc.tile_pool(name="vv", bufs=1) as vvp:
                xT = xtp.tile([128, 6, S], BF16, tag="xT")
                # v seq-major [sk, skpair, j, head, 65]; col 64 = ones
                vsl = vvp.tile([128, NPAIR, 2, LHEADS, D + 1], BF16, tag="v")
                nc.vector.memset(vsl[:, :, :, :, D : D + 1], 1.0)

                # --- x DMA, weights; transposes emitted lazily per group ---
                x_ap = x[:].rearrange("(n p) d -> p n d", p=128)
                x_ts = []
                for g in range(XG):
                    x_t = vvp.tile([128, NXG, HID], BF16, tag=f"x{g}", name=f"x_t{g}")
                    x_ts.append(x_t)
                nc.sync.dma_start(x_ts[0][:], x_ap[:, 0:NXG, :])
                for _p in range(PAIRS):
                    get_qk8(_p)
                wqk_r = wpool.tile([128, 6, 768], BF16, tag="wqkr")
                wqk_ap = wqk[:].rearrange("(c p) f -> p c f", p=128)
                for kc in range(6):
                    nc.sync.dma_start(
                        wqk_r[:, kc : kc + 1, :], wqk_ap[:, kc : kc + 1, :]
                    )
                for g in range(1, XG):
                    nc.sync.dma_start(
                        x_ts[g][:], x_ap[:, g * NXG : (g + 1) * NXG, :]
                    )
                wv_r = wpool.tile([128, 6, 384], BF16, tag="wvr")
                nc.sync.dma_start(
                    wv_r[:], wv[:].rearrange("(c p) f -> p c f", p=128)
                )
                wp_r = wpool.tile([128, 3, HID], BF16, tag="wpr")
                nc.sync.dma_start(
                    wp_r[:], wp[:].rearrange("(c p) f -> p c f", p=128)
                )

                def emit_xT_group(g):
                    # transposes: per hid-chunk j, 4 seq chunks into one
                    # psum tile, then one contiguous copy
                    for j in range(6):
                        tp = ps.tile([128, NXG, 128], BF16, tag="sc", bufs=3,
                                     name="tp")
                        for i in range(NXG):
                            nc.tensor.transpose(
                                tp[:, i, :],
                                x_ts[g][:, i, j * 128 : (j + 1) * 128],
                                ident[:],
                            )
                        dst = xT[:, j, g * NXG * 128 : (g + 1) * NXG * 128]
                        if (g * 6 + j) % 2 == 0:
                            nc.scalar.copy(dst, tp[:])
                        else:
                            nc.vector.tensor_copy(dst, tp[:])

                smp_holder = [None]

                def emit_qk_fillers(p, n):
                    """q+k for pair p block n as single-matmul thunks that
                    interleave between score units (keeps sc tiles flowing
                    to the exp engines)."""
                    qp = ps.tile([128, 2, 512], F32, tag="sc", bufs=3,
                                 name=f"qp_{p}_{n}")

                    def mk_mm(qk_i, wcol, kc):
                        def mm():
                            nc.tensor.matmul(
                                qp[:, qk_i, :],
                                wqk_r[:, kc, wcol * 128 : (wcol + 1) * 128],
                                xT[:, kc, n * 512 : (n + 1) * 512],
                                start=(kc == 0),
                                stop=(kc == 5),
                                skip_group_check=True,
                            )
                        return mm

                    fillers = []
                    for qk_i, wcol in ((0, p), (1, 3 + p)):
                        for kc in range(6):
                            fillers.append(mk_mm(qk_i, wcol, kc))

                    def drains():
                        sl = slice(n * 512, (n + 1) * 512)
                        q8_p, k8_p = get_qk8(p)
                        for dst8, qk_i, wcol in ((q8_p, 0, p), (k8_p, 1, 3 + p)):
                            for hi in range(2):
                                if (qk_i + hi) % 2 == 0:
                                    nc.vector.tensor_scalar(
                                        dst8[64 * hi : 64 * hi + 64, 0, sl],
                                        qp[64 * hi : 64 * hi + 64, qk_i, :],
                                        bqk_sb[64 * hi : 64 * hi + 64,
                                               wcol : wcol + 1],
                                        None,
                                        ADD,
                                    )
                                else:
                                    nc.scalar.activation(
                                        dst8[64 * hi : 64 * hi + 64, 0, sl],
                                        qp[64 * hi : 64 * hi + 64, qk_i, :],
                                        COPY_FN,
                                        bias=bqk_sb[64 * hi : 64 * hi + 64,
                                                    wcol : wcol + 1],
                                        scale=1.0,
                                    )
                    fillers.append(drains)
                    return fillers

                def emit_qk(p, n):
                    for f in emit_qk_fillers(p, n):
                        f()

                def emit_v(i):
                    """v for seq chunk i, all 6 heads, seq-major, bias-free."""
                    vp = ps.tile([128, 2, 512], F32, tag="sc", bufs=3)
                    for kc in range(6):
                        nc.tensor.matmul(
                            vp[:, 0, 0:384],
                            xT[:, kc, i * 128 : (i + 1) * 128],
                            wv_r[:, kc, :],
                            start=(kc == 0),
                            stop=(kc == 5),
                            skip_group_check=True,
                        )
                    nc.vector.tensor_copy(
                        vsl[:, i // 2, i % 2, :, 0:D],
                        vp[:, 0, 0:384].rearrange("p (h d) -> p h d", h=6),
                    )

                with (
                    tc.tile_pool(name="pt", bufs=1) as ptp,
                    tc.tile_pool(name="sm", bufs=1) as smp,
                ):
                    smp_holder[0] = smp
                    exp_idx = [0]

                    def emit_score_unit(p, n, pt, hi, i):
                        q8_p, k8_p = get_qk8(p)
                        sc = ps.tile([128, 2, 512], F32, tag="sc", bufs=3,
                                     name="sc")
                        for j in range(2):
                            sk = 2 * i + j
                            nc.tensor.matmul(
                                sc[:, j, :],
                                k8_p[64 * hi : 64 * hi + 64, :,
                                     sk * 128 : (sk + 1) * 128],
                                q8_p[64 * hi : 64 * hi + 64, :,
                                     n * 512 : (n + 1) * 512],
                                start=True,
                                stop=True,
                                perf_mode=DR,
                            )
                        dst = pt[:, 8 * hi + i, :, :]
                        if exp_idx[0] < 48:
                            eng = ("act", "act", "dve")[exp_idx[0] % 3]
                        else:
                            eng = EXP_PATTERN[exp_idx[0] % len(EXP_PATTERN)]
                        exp_idx[0] += 1
                        if eng == "act":
                            nc.scalar.activation(
                                dst, sc[:], EXP, bias=0.0, scale=0.125
                            )
                        else:
                            nc.vector.tensor_scalar(
                                dst.bitcast(U16),
                                sc[:],
                                SCHR_MUL, SCHR_ADD, MULT, ADD,
                            )

                    def emit_scores(p, n, pt, hook=None, fillers=None):
                        fillers = list(fillers or [])
                        u = 0
                        for hi in range(2):
                            for i in range(NPAIR):
                                emit_score_unit(p, n, pt, hi, i)
                                # dispatch filler PE work between units so
                                # sc tiles keep flowing to the exp engines
                                units_left = 16 - u
                                share = -(-len(fillers) // units_left)
                                for _ in range(share):
                                    fillers.pop(0)()
                                if hook is not None and u % 4 == 3:
                                    hook(u // 4)
                                u += 1
                        for f in fillers:
                            f()

                    ao_ns = {}

                    def emit_av_chunk(p, n, pt, c):
                            av = ps.tile([128, 2, D + 1], F32, tag="av", bufs=2)
                            first = True
                            for hi in range(2):
                                for i in range(NPAIR):
                                    for j in range(2):
                                        nc.tensor.matmul(
                                            av[:, hi, :],
                                            pt[:, 8 * hi + i, j,
                                               c * 128 : (c + 1) * 128],
                                            vsl[:, i, j, 2 * p + hi, :],
                                            start=first,
                                            stop=(hi == 1 and i == NPAIR - 1
                                                  and j == 1),
                                            skip_group_check=True,
                                        )
                                        first = False
                            rec = smp.tile([128, 2], F32, tag="rec", bufs=4)
                            nc.vector.reciprocal(rec[:], av[:, :, D])
                            ao_n = smp.tile([128, 2, D], BF16, tag="aon", bufs=32,
                                            name=f"ao_n_{p}_{n}_{c}")
                            nc.vector.tensor_tensor(
                                ao_n[:],
                                av[:, :, 0:D],
                                rec[:].unsqueeze(2).broadcast_to([128, 2, D]),
                                MULT,
                            )
                            ao_ns[(p, n, c)] = ao_n

                    def emit_av(p, n, pt):
                        for c in range(4):
                            emit_av_chunk(p, n, pt, c)

                    def emit_aoT_block(p, n):
                        # transpose one block's normalized outputs into aoT
                        tp2 = ps.tile([128, 4, 128], BF16, tag="av",
                                      bufs=2, name=f"tp2_{p}_{n}")
                        for c in range(4):
                            nc.tensor.transpose(
                                tp2[:, c, :],
                                ao_ns.pop((p, n, c))[:].rearrange(
                                    "p h d -> p (h d)"),
                                ident[:],
                            )
                        nc.vector.tensor_copy(
                            aoT[:, p, n * 512 : (n + 1) * 512],
                            tp2[:].rearrange("p a b -> p (a b)"),
                        )

                    def emit_proj_block(n):
                        for m in range(6):
                            pp = ps.tile([128, 2, 512], F32, tag="sc", bufs=3,
                                         name="pp")
                            for kc in range(3):
                                nc.tensor.matmul(
                                    pp[:, 0, :],
                                    wp_r[:, kc, m * 128 : (m + 1) * 128],
                                    aoT[:, kc, n * 512 : (n + 1) * 512],
                                    start=(kc == 0),
                                    stop=(kc == 2),
                                    skip_group_check=True,
                                )
                            yt_t = smp.tile([128, 512], F32, tag="yT", bufs=6, name="yt_t")
                            if m % 2 == 0:
                                nc.scalar.copy(yt_t[:], pp[:, 0, :])
                            else:
                                nc.vector.tensor_copy(yt_t[:], pp[:, 0, :])
                            nc.sync.dma_start(
                                yT_ap[:, m, n * 512 : (n + 1) * 512], yt_t[:]
                            )

                    if taps:
                        def tap(dram, ap):
                            nc.sync.dma_start(dram[:], ap)
                    pt_tiles = {}
                    prev = None
                    aoT_pending = []
                    for p in range(PAIRS):
                        for n in range(NBLK):
                            pt = ptp.tile([128, 16, 2, 512], BF16, tag="pt",
                                          bufs=2, name=f"pt_{p}_{n}")
                            pt_tiles[(p, n)] = pt
                            # interleave prev block's AV chunks into this
                            # block's score stream (safe once v is complete)
                            hook = None
                            if prev is not None and not (p == 0 and n <= 1):
                                pp_, pn_, ppt_ = prev
                                hook = lambda c: emit_av_chunk(pp_, pn_, ppt_, c)
                            if p == 0 and n == 0:
                                # xT arrives per x-group; k for the whole
                                # pair arrives per qk block; emit score
                                # units as their sk range lands
                                for kb in range(NBLK):
                                    emit_xT_group(kb)
                                    emit_qk(0, kb)
                                    for i in (2 * kb, 2 * kb + 1):
                                        for hi in range(2):
                                            emit_score_unit(0, 0, pt, hi, i)
                                emit_qk(1, 0)
                            else:
                                fillers = []
                                if p < PAIRS - 1:
                                    fillers += emit_qk_fillers(p + 1, n)
                                if p == 0 and n == 1:
                                    for i in range(16):
                                        fillers.append(
                                            lambda i=i: emit_v(i))
                                emit_scores(p, n, pt, hook=hook,
                                            fillers=fillers)
                            if taps and p == 0 and n == 0:
                                tap(t_xT, xT[:].rearrange("p a b -> p (a b)"))
                                tap(t_q8, q8_tiles[0][:].rearrange("p b c -> p (b c)"))
                                tap(t_k8, k8_tiles[0][:].rearrange("p b c -> p (b c)"))
                                tap(t_v, vsl[:].rearrange("p a b c d -> p (a b c d)"))
                                tap(t_pt, pt[:].rearrange("p a b c -> p (a b c)"))
                            if prev is not None and hook is None:
                                emit_av(*prev)
                            if prev is not None:
                                aoT_pending.append(prev[:2])
                            # emit one pending aoT block, one block lagged
                            # so its norms have drained
                            if len(aoT_pending) >= 2:
                                key = aoT_pending.pop(0)
                                emit_aoT_block(*key)
                                if key[0] == PAIRS - 1:
                                    emit_proj_block(key[1])
                            prev = (p, n, pt)
                    # drain already-complete aoT blocks (and their proj)
                    # before the last block's AV to shorten the tail
                    for key in aoT_pending:
                        emit_aoT_block(*key)
                        if key[0] == PAIRS - 1:
                            emit_proj_block(key[1])
                    emit_av(*prev)
                    emit_aoT_block(*prev[:2])
                    emit_proj_block(prev[1])
                    if taps:
                        tap(t_ao, aoT[:].rearrange("p a b -> p (a b)"))

    nc.finalize()
    return nc


_NC_CACHE = {}


def _get_nc(S):
    if S not in _NC_CACHE:
        _NC_CACHE[S] = build_nc(S)
    return _NC_CACHE[S]


def kernel(x, qkv_w, qkv_b, proj_w, proj_b, return_res=False, **run_kwargs):
    import ml_dtypes

    x = np.asarray(x, dtype=np.float32)
    qkv_w = np.asarray(qkv_w, dtype=np.float32)
    qkv_b = np.asarray(qkv_b, dtype=np.float32)
    proj_w = np.asarray(proj_w, dtype=np.float32)
    proj_b = np.asarray(proj_b, dtype=np.float32)
    B, S, _ = x.shape

    nc = _get_nc(S)
    bf = ml_dtypes.bfloat16
    x_bf = x.astype(bf)
    in_maps = []
    for c in range(8):
        b, g = c // 2, c % 2
        qs = slice(384 * g, 384 * g + 384)
        ks = slice(768 + 384 * g, 768 + 384 * g + 384)
        vs = slice(1536 + 384 * g, 1536 + 384 * g + 384)
        in_maps.append(
            {
                "x": np.ascontiguousarray(x_bf[b]).view(np.uint16),
                "wqk": np.ascontiguousarray(
                    np.concatenate([qkv_w[:, qs], qkv_w[:, ks]], axis=1).astype(bf)
                ).view(np.uint16),
                "wv": np.ascontiguousarray(qkv_w[:, vs].astype(bf)).view(np.uint16),
                "bqk": np.ascontiguousarray(
                    np.concatenate([qkv_b[qs], qkv_b[ks]])
                ),
                "wp": np.ascontiguousarray(
                    proj_w[384 * g : 384 * g + 384, :].astype(bf)
                ).view(np.uint16),
                "z8": np.zeros((128, 2048), np.uint8),
            }
        )
    try:
        res = run_bass_kernel_spmd(
            nc, in_maps, core_ids=list(range(8)), **run_kwargs
        )
    except Exception:
        # transient NRT/device errors happen occasionally; retry once
        res = run_bass_kernel_spmd(
            nc, in_maps, core_ids=list(range(8)), **run_kwargs
        )
    # effective bias: the v bias passes through softmax additively
    b_eff = (proj_b.astype(np.float64)
             + qkv_b[1536:].astype(np.float64) @ proj_w.astype(np.float64)
             ).astype(np.float32)
    out = np.empty((B, S, HID), np.float32)
    for b in range(B):
        yt = res.results[2 * b]["yT"] + res.results[2 * b + 1]["yT"]
        out[b] = yt.T + b_eff
    if return_res:
        return out, res
    return out


# revision 51
# speedup vs baseline: 1.0015x; 1.0015x over previous
"""MultiHeadAttention Trainium2 kernel (8 NeuronCores).

Sharding: core c -> (batch b = c//2, head-group g = c%2) of the 12 heads.
Each core computes attention for its 6 heads of one batch element and a
partial projection; the host sums the two head-group partials per batch
element and adds the effective proj bias (proj_b + bv @ proj_w; the v bias
is additive after softmax because attention rows sum to 1).

Per-core dataflow (bf16 datapath, fp8 DoubleRow scores):
  x bf16 [S,768] --PE-transpose--> xT bf16 [128,6,S]
  q/k psum f32 = wqk_bf16^T @ xT; DVE adds bias and converts to fp8 in a
    zero-padded DoubleRow layout q8/k8 [64|64 part, pair, 2, S]
  v bf16 seq-major vsl [sk, skpair, j, head, 65] (+ ones col for denom)
  scores[sk,sq] = DoubleRow fp8 matmul (contraction 64 + 64 zeros)
  pt = exp(scores/8): split ACT (exact) / Pool / DVE (Schraudolph bits)
  av[sq,2,65] += pt-chunk^T @ v    (bf16, psum accum; col 64 = denom)
  ao_n = av / denom (broadcast divide, DVE), PE-transpose to aoT [384,S]
  yT[768,S] = wp_bf16^T @ aoT      (partial projection, host sums pairs)
"""
import sys

sys.path.insert(0, "/opt/trn_rl_repo")

import numpy as np

import concourse.bass as bass
import concourse.mybir as mybir
import concourse.tile as tile
from concourse import bacc
from concourse.bass_utils import run_bass_kernel_spmd
from concourse.masks import make_identity

F32 = mybir.dt.float32
BF16 = mybir.dt.bfloat16
U16 = mybir.dt.uint16
FP8 = mybir.dt.float8e4
EXP = mybir.ActivationFunctionType.Exp
COPY_FN = mybir.ActivationFunctionType.Identity
ADD = mybir.AluOpType.add
MULT = mybir.AluOpType.mult
DIV = mybir.AluOpType.divide
DR = mybir.MatmulPerfMode.DoubleRow

HID = 768
D = 64  # head dim
LHEADS = 6  # heads per core
PAIRS = 3

LOG2E = 1.4426950408889634
# Schraudolph-in-bf16-bits: n = x*0.125*128*log2e + (16256 + c); floor via
# the executor's f32->u16 cast. c=-6.85 calibrated for min rms vs exp().
SCHR_MUL = 0.125 * 128.0 * LOG2E
SCHR_ADD = 16256.0 - 6.85

# exp engine split: per block of 16 exp units (1024 rows each).
# GPSIMD (Pool) cannot access PSUM on TRN2, so only ACT and DVE apply.
EXP_PATTERN = (
    "act", "dve", "act", "act", "dve", "act", "dve", "act",
    "act", "dve", "act", "dve", "act", "act", "dve", "act",
)


def build_nc(S: int, taps: bool = False):
    nc = bacc.Bacc("TRN2", target_bir_lowering=False, debug=False)
    NSEQ = S // 128  # seq chunks of 128
    NBLK = S // 512  # seq blocks of 512
    NPAIR = NSEQ // 2  # sk chunk pairs
    XG = 4  # x DMA chunk groups
    NXG = NSEQ // XG

    x = nc.dram_tensor("x", [S, HID], BF16, kind="ExternalInput")
    wqk = nc.dram_tensor("wqk", [HID, 768], BF16, kind="ExternalInput")
    wv = nc.dram_tensor("wv", [HID, 384], BF16, kind="ExternalInput")
    bqk = nc.dram_tensor("bqk", [768], F32, kind="ExternalInput")
    wp = nc.dram_tensor("wp", [384, HID], BF16, kind="ExternalInput")
    z8 = nc.dram_tensor("z8", [128, 2048], FP8, kind="ExternalInput")
    yT = nc.dram_tensor("yT", [HID, S], F32, kind="ExternalOutput")
    if taps:
        t_xT = nc.dram_tensor("t_xT", [128, 6 * S], BF16, kind="ExternalOutput")
        t_q8 = nc.dram_tensor("t_q8", [128, 2 * S], FP8, kind="ExternalOutput")
        t_k8 = nc.dram_tensor("t_k8", [128, 2 * S], FP8, kind="ExternalOutput")
        t_v = nc.dram_tensor("t_v", [128, NSEQ // 2 * 2 * 6 * (D + 1)], BF16, kind="ExternalOutput")
        t_pt = nc.dram_tensor("t_pt", [128, 16 * 2 * 512], BF16, kind="ExternalOutput")
        t_ao = nc.dram_tensor("t_ao", [128, PAIRS * S], BF16, kind="ExternalOutput")
        t_aon = nc.dram_tensor("t_aon", [128, 2 * D], BF16, kind="ExternalOutput")
        t_qp = nc.dram_tensor("t_qp", [128, 2, 512], F32, kind="ExternalOutput")

    with tile.TileContext(nc) as tc:
        with (
            tc.tile_pool(name="const", bufs=1) as cp,
            tc.tile_pool(name="wts", bufs=1) as wpool,
            tc.tile_pool(name="qk8", bufs=1) as qk8p,
            tc.tile_pool(name="ao", bufs=1) as aop,
            tc.tile_pool(name="ps", bufs=1, space="PSUM") as ps,
        ):
            identf = cp.tile([128, 128], F32, tag="identf")
            make_identity(nc, identf[:])
            ident = cp.tile([128, 128], BF16, tag="ident")
            nc.vector.tensor_copy(ident[:], identf[:])
            # q/k bias, feature-major [128, 6] (chunk c: c<3 q, c>=3 k)
            bqk_sb = cp.tile([128, 6], F32, tag="bqk")
            nc.sync.dma_start(bqk_sb[:], bqk[:].rearrange("(c p) -> p c", p=128))
            # load the exp ACT table off the critical path
            warm = cp.tile([1, 16], F32, tag="warm")
            nc.vector.memset(warm[:], 1.0)
            nc.scalar.activation(warm[:], warm[:], EXP, bias=0.0, scale=0.0)

            # fp8 q/k, zero-padded DoubleRow layout, one tile per pair:
            # [128, 2, S]; partition half = head-of-pair, dim1 = j (j=1
            # stays zero) so [64*hi:64*hi+64, :, a:b] is a DR operand.
            q8_tiles = {}
            k8_tiles = {}

            def get_qk8(p):
                if p not in q8_tiles:
                    q8_p = qk8p.tile([128, 2, S], FP8, tag="q8", bufs=3,
                                     name=f"q8_{p}")
                    k8_p = qk8p.tile([128, 2, S], FP8, tag="k8", bufs=3,
                                     name=f"k8_{p}")
                    nc.sync.dma_start(q8_p[:, 1, :], z8[:])
                    nc.sync.dma_start(k8_p[:, 1, :], z8[:])
                    q8_tiles[p] = q8_p
                    k8_tiles[p] = k8_p
                return q8_tiles[p], k8_tiles[p]

            aoT = aop.tile([128, PAIRS, S], BF16, tag="aoT")
            yT_ap = yT[:].rearrange("(c p) s -> p c s", p=128)

            with tc.tile_pool(name="xT", bufs=1) as xtp, \
                 tc.tile_pool(name="vv", bufs=1) as vvp:
                xT = xtp.tile([128, 6, S], BF16, tag="xT")
                # v seq-major [sk, skpair, j, head, 65]; col 64 = ones
                vsl = vvp.tile([128, NPAIR, 2, LHEADS, D + 1], BF16, tag="v")
                nc.vector.memset(vsl[:, :, :, :, D : D + 1], 1.0)

                # --- x DMA, weights; transposes emitted lazily per group ---
                x_ap = x[:].rearrange("(n p) d -> p n d", p=128)
                x_ts = []
                for g in range(XG):
                    x_t = vvp.tile([128, NXG, HID], BF16, tag=f"x{g}", name=f"x_t{g}")
                    x_ts.append(x_t)
                nc.sync.dma_start(x_ts[0][:], x_ap[:, 0:NXG, :])
                for _p in range(PAIRS):
                    get_qk8(_p)
                wqk_r = wpool.tile([128, 6, 768], BF16, tag="wqkr")
                wqk_ap = wqk[:].rearrange("(c p) f -> p c f", p=128)
                for kc in range(6):
                    nc.sync.dma_start(
                        wqk_r[:, kc : kc + 1, :], wqk_ap[:, kc : kc + 1, :]
                    )
                for g in range(1, XG):
                    nc.sync.dma_start(
                        x_ts[g][:], x_ap[:, g * NXG : (g + 1) * NXG, :]
                    )
                wv_r = wpool.tile([128, 6, 384], BF16, tag="wvr")
                nc.sync.dma_start(
                    wv_r[:], wv[:].rearrange("(c p) f -> p c f", p=128)
                )
                wp_r = wpool.tile([128, 3, HID], BF16, tag="wpr")
                nc.sync.dma_start(
                    wp_r[:], wp[:].rearrange("(c p) f -> p c f", p=128)
                )

                def emit_xT_group(g):
                    # transposes: per hid-chunk j, 4 seq chunks into one
                    # psum tile, then one contiguous copy
                    for j in range(6):
                        tp = ps.tile([128, NXG, 128], BF16, tag="sc", bufs=3,
                                     name="tp")
                        for i in range(NXG):
                            nc.tensor.transpose(
                                tp[:, i, :],
                                x_ts[g][:, i, j * 128 : (j + 1) * 128],
                                ident[:],
                            )
                        dst = xT[:, j, g * NXG * 128 : (g + 1) * NXG * 128]
                        if (g * 6 + j) % 2 == 0:
                            nc.scalar.copy(dst, tp[:])
                        else:
                            nc.vector.tensor_copy(dst, tp[:])

                smp_holder = [None]

                def emit_qk_fillers(p, n):
                    """q+k for pair p block n as single-matmul thunks that
                    interleave between score units (keeps sc tiles flowing
                    to the exp engines)."""
                    qp = ps.tile([128, 2, 512], F32, tag="sc", bufs=3,
                                 name=f"qp_{p}_{n}")

                    def mk_mm(qk_i, wcol, kc):
                        def mm():
                            nc.tensor.matmul(
                                qp[:, qk_i, :],
                                wqk_r[:, kc, wcol * 128 : (wcol + 1) * 128],
                                xT[:, kc, n * 512 : (n + 1) * 512],
                                start=(kc == 0),
                                stop=(kc == 5),
                                skip_group_check=True,
                            )
                        return mm

                    fillers = []
                    for qk_i, wcol in ((0, p), (1, 3 + p)):
                        for kc in range(6):
                            fillers.append(mk_mm(qk_i, wcol, kc))

                    def drains():
                        sl = slice(n * 512, (n + 1) * 512)
                        q8_p, k8_p = get_qk8(p)
                        for dst8, qk_i, wcol in ((q8_p, 0, p), (k8_p, 1, 3 + p)):
                            for hi in range(2):
                                if (qk_i + hi) % 2 == 0:
                                    nc.vector.tensor_scalar(
                                        dst8[64 * hi : 64 * hi + 64, 0, sl],
                                        qp[64 * hi : 64 * hi + 64, qk_i, :],
                                        bqk_sb[64 * hi : 64 * hi + 64,
                                               wcol : wcol + 1],
                                        None,
                                        ADD,
                                    )
                                else:
                                    nc.scalar.activation(
                                        dst8[64 * hi : 64 * hi + 64, 0, sl],
                                        qp[64 * hi : 64 * hi + 64, qk_i, :],
                                        COPY_FN,
                                        bias=bqk_sb[64 * hi : 64 * hi + 64,
                                                    wcol : wcol + 1],
                                        scale=1.0,
                                    )
                    fillers.append(drains)
                    return fillers

                def emit_qk(p, n):
                    for f in emit_qk_fillers(p, n):
                        f()

                def emit_v(i):
                    """v for seq chunk i, all 6 heads, seq-major, bias-free."""
                    vp = ps.tile([128, 2, 512], F32, tag="sc", bufs=3)
                    for kc in range(6):
                        nc.tensor.matmul(
                            vp[:, 0, 0:384],
                            xT[:, kc, i * 128 : (i + 1) * 128],
                            wv_r[:, kc, :],
                            start=(kc == 0),
                            stop=(kc == 5),
                            skip_group_check=True,
                        )
                    nc.vector.tensor_copy(
                        vsl[:, i // 2, i % 2, :, 0:D],
                        vp[:, 0, 0:384].rearrange("p (h d) -> p h d", h=6),
                    )

                with (
                    tc.tile_pool(name="pt", bufs=1) as ptp,
                    tc.tile_pool(name="sm", bufs=1) as smp,
                ):
                    smp_holder[0] = smp
                    exp_idx = [0]

                    def emit_score_unit(p, n, pt, hi, i):
                        q8_p, k8_p = get_qk8(p)
                        sc = ps.tile([128, 2, 512], F32, tag="sc", bufs=3,
                                     name="sc")
                        for j in range(2):
                            sk = 2 * i + j
                            nc.tensor.matmul(
                                sc[:, j, :],
                                k8_p[64 * hi : 64 * hi + 64, :,
                                     sk * 128 : (sk + 1) * 128],
                                q8_p[64 * hi : 64 * hi + 64, :,
                                     n * 512 : (n + 1) * 512],
                                start=True,
                                stop=True,
                                perf_mode=DR,
                            )
                        dst = pt[:, 8 * hi + i, :, :]
                        if exp_idx[0] < 64:
                            eng = ("act", "act", "dve")[exp_idx[0] % 3]
                        else:
                            eng = EXP_PATTERN[exp_idx[0] % len(EXP_PATTERN)]
                        exp_idx[0] += 1
                        if eng == "act":
                            nc.scalar.activation(
                                dst, sc[:], EXP, bias=0.0, scale=0.125
                            )
                        else:
                            nc.vector.tensor_scalar(
                                dst.bitcast(U16),
                                sc[:],
                                SCHR_MUL, SCHR_ADD, MULT, ADD,
                            )

                    def emit_scores(p, n, pt, hook=None, fillers=None):
                        fillers = list(fillers or [])
                        u = 0
                        for hi in range(2):
                            for i in range(NPAIR):
                                emit_score_unit(p, n, pt, hi, i)
                                # dispatch filler PE work between units so
                                # sc tiles keep flowing to the exp engines
                                units_left = 16 - u
                                share = -(-len(fillers) // units_left)
                                for _ in range(share):
                                    fillers.pop(0)()
                                if hook is not None and u % 4 == 3:
                                    hook(u // 4)
                                u += 1
                        for f in fillers:
                            f()

                    ao_ns = {}

                    def emit_av_chunk(p, n, pt, c):
                            av = ps.tile([128, 2, D + 1], F32, tag="av", bufs=2)
                            first = True
                            for hi in range(2):
                                for i in range(NPAIR):
                                    for j in range(2):
                                        nc.tensor.matmul(
                                            av[:, hi, :],
                                            pt[:, 8 * hi + i, j,
                                               c * 128 : (c + 1) * 128],
                                            vsl[:, i, j, 2 * p + hi, :],
                                            start=first,
                                            stop=(hi == 1 and i == NPAIR - 1
                                                  and j == 1),
                                            skip_group_check=True,
                                        )
                                        first = False
                            rec = smp.tile([128, 2], F32, tag="rec", bufs=4)
                            nc.vector.reciprocal(rec[:], av[:, :, D])
                            ao_n = smp.tile([128, 2, D], BF16, tag="aon", bufs=32,
                                            name=f"ao_n_{p}_{n}_{c}")
                            nc.vector.tensor_tensor(
                                ao_n[:],
                                av[:, :, 0:D],
                                rec[:].unsqueeze(2).broadcast_to([128, 2, D]),
                                MULT,
                            )
                            ao_ns[(p, n, c)] = ao_n

                    def emit_av(p, n, pt):
                        for c in range(4):
                            emit_av_chunk(p, n, pt, c)

                    def emit_aoT_block(p, n):
                        # transpose one block's normalized outputs into aoT
                        tp2 = ps.tile([128, 4, 128], BF16, tag="av",
                                      bufs=2, name=f"tp2_{p}_{n}")
                        for c in range(4):
                            nc.tensor.transpose(
                                tp2[:, c, :],
                                ao_ns.pop((p, n, c))[:].rearrange(
                                    "p h d -> p (h d)"),
                                ident[:],
                            )
                        nc.vector.tensor_copy(
                            aoT[:, p, n * 512 : (n + 1) * 512],
                            tp2[:].rearrange("p a b -> p (a b)"),
                        )

                    def emit_proj_block(n):
                        for m in range(6):
                            pp = ps.tile([128, 2, 512], F32, tag="sc", bufs=3,
                                         name="pp")
                            for kc in range(3):
                                nc.tensor.matmul(
                                    pp[:, 0, :],
                                    wp_r[:, kc, m * 128 : (m + 1) * 128],
                                    aoT[:, kc, n * 512 : (n + 1) * 512],
                                    start=(kc == 0),
                                    stop=(kc == 2),
                                    skip_group_check=True,
                                )
                            yt_t = smp.tile([128, 512], F32, tag="yT", bufs=6, name="yt_t")
                            if m % 2 == 0:
                                nc.scalar.copy(yt_t[:], pp[:, 0, :])
                            else:
                                nc.vector.tensor_copy(yt_t[:], pp[:, 0, :])
                            nc.sync.dma_start(
                                yT_ap[:, m, n * 512 : (n + 1) * 512], yt_t[:]
                            )

                    if taps:
                        def tap(dram, ap):
                            nc.sync.dma_start(dram[:], ap)
                    pt_tiles = {}
                    prev = None
                    aoT_pending = []
                    for p in range(PAIRS):
                        for n in range(NBLK):
                            pt = ptp.tile([128, 16, 2, 512], BF16, tag="pt",
                                          bufs=2, name=f"pt_{p}_{n}")
                            pt_tiles[(p, n)] = pt
                            # interleave prev block's AV chunks into this
                            # block's score stream (safe once v is complete)
                            hook = None
                            if prev is not None and not (p == 0 and n <= 1):
                                pp_, pn_, ppt_ = prev
                                hook = lambda c: emit_av_chunk(pp_, pn_, ppt_, c)
                            if p == 0 and n == 0:
                                # xT arrives per x-group; k for the whole
                                # pair arrives per qk block; emit score
                                # units as their sk range lands
                                for kb in range(NBLK):
                                    emit_xT_group(kb)
                                    emit_qk(0, kb)
                                    for i in (2 * kb, 2 * kb + 1):
                                        for hi in range(2):
                                            emit_score_unit(0, 0, pt, hi, i)
                                emit_qk(1, 0)
                            else:
                                fillers = []
                                if p < PAIRS - 1:
                                    fillers += emit_qk_fillers(p + 1, n)
                                if p == 0 and n == 1:
                                    for i in range(16):
                                        fillers.append(
                                            lambda i=i: emit_v(i))
                                emit_scores(p, n, pt, hook=hook,
                                            fillers=fillers)
                            if taps and p == 0 and n == 0:
                                tap(t_xT, xT[:].rearrange("p a b -> p (a b)"))
                                tap(t_q8, q8_tiles[0][:].rearrange("p b c -> p (b c)"))
                                tap(t_k8, k8_tiles[0][:].rearrange("p b c -> p (b c)"))
                                tap(t_v, vsl[:].rearrange("p a b c d -> p (a b c d)"))
                                tap(t_pt, pt[:].rearrange("p a b c -> p (a b c)"))
                            if prev is not None and hook is None:
                                emit_av(*prev)
                            if prev is not None:
                                aoT_pending.append(prev[:2])
                            # emit one pending aoT block, one block lagged
                            # so its norms have drained
                            if len(aoT_pending) >= 2:
                                key = aoT_pending.pop(0)
                                emit_aoT_block(*key)
                                if key[0] == PAIRS - 1:
                                    emit_proj_block(key[1])
                            prev = (p, n, pt)
                    # drain already-complete aoT blocks (and their proj)
                    # before the last block's AV to shorten the tail
                    for key in aoT_pending:
                        emit_aoT_block(*key)
                        if key[0] == PAIRS - 1:
                            emit_proj_block(key[1])
                    emit_av(*prev)
                    emit_aoT_block(*prev[:2])
                    emit_proj_block(prev[1])
                    if taps:
                        tap(t_ao, aoT[:].rearrange("p a b -> p (a b)"))

    nc.finalize()
    return nc


_NC_CACHE = {}


def _get_nc(S):
    if S not in _NC_CACHE:
        _NC_CACHE[S] = build_nc(S)
    return _NC_CACHE[S]


def kernel(x, qkv_w, qkv_b, proj_w, proj_b, return_res=False, **run_kwargs):
    import ml_dtypes

    x = np.asarray(x, dtype=np.float32)
    qkv_w = np.asarray(qkv_w, dtype=np.float32)
    qkv_b = np.asarray(qkv_b, dtype=np.float32)
    proj_w = np.asarray(proj_w, dtype=np.float32)
    proj_b = np.asarray(proj_b, dtype=np.float32)
    B, S, _ = x.shape

    nc = _get_nc(S)
    bf = ml_dtypes.bfloat16
    x_bf = x.astype(bf)
    in_maps = []
    for c in range(8):
        b, g = c // 2, c % 2
        qs = slice(384 * g, 384 * g + 384)
        ks = slice(768 + 384 * g, 768 + 384 * g + 384)
        vs = slice(1536 + 384 * g, 1536 + 384 * g + 384)
        in_maps.append(
            {
                "x": np.ascontiguousarray(x_bf[b]).view(np.uint16),
                "wqk": np.ascontiguousarray(
                    np.concatenate([qkv_w[:, qs], qkv_w[:, ks]], axis=1).astype(bf)
                ).view(np.uint16),
                "wv": np.ascontiguousarray(qkv_w[:, vs].astype(bf)).view(np.uint16),
                "bqk": np.ascontiguousarray(
                    np.concatenate([qkv_b[qs], qkv_b[ks]])
                ),
                "wp": np.ascontiguousarray(
                    proj_w[384 * g : 384 * g + 384, :].astype(bf)
                ).view(np.uint16),
                "z8": np.zeros((128, 2048), np.uint8),
            }
        )
    try:
        res = run_bass_kernel_spmd(
            nc, in_maps, core_ids=list(range(8)), **run_kwargs
        )
    except Exception:
        # transient NRT/device errors happen occasionally; retry once
        res = run_bass_kernel_spmd(
            nc, in_maps, core_ids=list(range(8)), **run_kwargs
        )
    # effective bias: the v bias passes through softmax additively
    b_eff = (proj_b.astype(np.float64)
             + qkv_b[1536:].astype(np.float64) @ proj_w.astype(np.float64)
             ).astype(np.float32)
    out = np.empty((B, S, HID), np.float32)
    for b in range(B):
        yt = res.results[2 * b]["yT"] + res.results[2 * b + 1]["yT"]
        out[b] = yt.T + b_eff
    if return_res:
        return out, res
    return out
